# revision 2
# baseline (speedup 1.0000x reference)
"""Trainium2 Bass kernel for nn_ODEG_8942121911067 (gnn_message_passing).

Math (the reference Euler loop collapses to its last step, f constant):

    out = relu(q + a),  a = 0.125*sigmoid(alpha)_i * (adj @ x_aug)
    q   = 0.5*x_aug + 0.25*S*R + 0.25*(x_aug @_t W2mix)

with x_aug = concat([x, zeros10], -1), S[b,n,t] = sum_f x_aug[b,n,t,f],
R[m] = sum_n ((w*clip(d,0,1)) @ w.T)[m,n], W2mix = (w2*clip(d2,0,1)) @ w2.T.

Device strategy (data-parallel over batch, 4 batches/core on 8 cores).
The device computes the 26 GFLOP adjacency message-passing term
`a = A @ x` in fp8 (A^T pre-scaled 2^20 on host, result scaled 2^13);
the precision-critical linear terms stay in host fp32 and the output is
assembled as relu(q + 2^-13 * a) in numpy.

v1 schedule (coarse DMA, per-batch stores):
  - 6 load dispatches on sync: adj [128,4,512], x b0 split in two
    k-tile halves (so the PE can start after 0.65 MB instead of 1.05),
    then one whole [128,4,1536] tile per remaining batch.
  - 12 HAM-warmup matmuls hold the PE activity window open during the
    load lead-in so the 1.2->2.4 GHz clock ramp happens off the
    critical path.
  - PSUM as [128,1536] 3-bank tiles (pool of 2 + warmup bank): one
    matmul accumulation group per output (b,ic) tile, evicted in ONE
    DVE/ACT scaled-copy per ic (alternating engines so neither gates).
  - Output per batch in one [128,4,1536] SBUF tile; stores are 1
    dispatch each for b0-b2 (alternating gpsimd/scalar) and 2 for b3
    (split so the tail after the last matmul is one 0.59+0.20 MB pair
    on idle engines instead of a serial whole-batch store).
  - HBM traffic/core: 3.15 MB x + 0.26 MB adj in, 3.15 MB a out; the
    96 DoubleRow fp8 matmuls (~216 ns each warm) are the roofline.
"""

import numpy as np

B, N, T, F = 32, 512, 24, 64
NUM_ZEROS = 10
FA = F + NUM_ZEROS  # 74
N_CORES = 8
BPC = B // N_CORES  # batches per core = 4
NT = N // 128  # node chunks = 4
TF = T * F  # 1536
SCALE_AT = 2.0 ** 20  # fp8 subnormal-avoidance scale on the stationary
SCALE_A = 2.0 ** 13  # scale of the returned adjacency term
EVICT = SCALE_A / SCALE_AT  # 2^-7, applied at PSUM eviction

_CACHE = {}


def _build():
    import concourse.mybir as mybir
    import concourse.tile as tile
    from concourse import bacc

    fp8 = mybir.dt.float8e4
    f32 = mybir.dt.float32

    nc = bacc.Bacc("TRN2", target_bir_lowering=False, debug=False,
                   num_devices=N_CORES)
    x_d = nc.dram_tensor("xin", [BPC, N, T, F], fp8, kind="ExternalInput").ap()
    at_d = nc.dram_tensor("at", [N, N], fp8, kind="ExternalInput").ap()
    out_d = nc.dram_tensor("out", [BPC, N, T, F], fp8,
                           kind="ExternalOutput").ap()

    with tile.TileContext(nc) as tc:
        with (
            tc.tile_pool(name="const", bufs=1) as cpool,
            tc.tile_pool(name="xp", bufs=5) as xpool,
            tc.tile_pool(name="op", bufs=4) as opool,
            tc.tile_pool(name="ps", bufs=2, space="PSUM") as pspool,
            tc.tile_pool(name="wp", bufs=1, space="PSUM") as wpool,
        ):
            # ---- loads: all on sync, first-use order, coarse ----
            atile = cpool.tile([128, NT, N], fp8, tag="at")
            nc.sync.dma_start(
                atile[:], at_d[:].rearrange("(c p) n -> p c n", p=128))
            # b0 split into k-tile halves so the first matmul group can
            # start after adj + 0.39 MB instead of adj + 0.79 MB.
            xb0 = []
            for h in range(2):
                xh = xpool.tile([128, 2, TF], fp8, tag="xt",
                                name=f"x0_{h}")
                nc.sync.dma_start(
                    xh[:], x_d[0, h * 256:(h + 1) * 256].rearrange(
                        "(c p) t f -> p c (t f)", p=128))
                xb0.append(xh)
            xts = []
            for b in range(1, BPC):
                xt = xpool.tile([128, NT, TF], fp8, tag="xt",
                                name=f"x{b}")
                nc.sync.dma_start(
                    xt[:], x_d[b].rearrange("(c p) t f -> p c (t f)", p=128))
                xts.append(xt)

            def rhs(b, kp):
                if b == 0:
                    return xb0[kp][:]
                return xts[b - 1][:, 2 * kp:2 * kp + 2]

            # ---- HAM warmup: hold the PE activity window open ----
            wmov = cpool.tile([128, 512], fp8, tag="wmov")
            nc.vector.memset(wmov[:], 0)
            wps = wpool.tile([128, 512], f32, tag="wps", name="wps")
            for _ in range(12):
                nc.tensor.matmul(wps[:], wmov[:, 0:128], wmov[:],
                                 start=True, stop=True)

            # ---- main stream ----
            ev = 0
            NCH = TF // 512  # 3 psum banks per (b, ic) accumulation group
            otiles = []
            for b in range(BPC):
                ot = opool.tile([128, NT, TF], fp8, tag="ot", name=f"o{b}")
                otiles.append(ot)
                for ic in range(NT):
                    mcol = slice(ic * 128, (ic + 1) * 128)
                    ps = pspool.tile([128, TF], f32, tag="ps",
                                     name=f"ps_{b}_{ic}")
                    for kp in range(2):
                        for nch in range(NCH):
                            ccol = slice(nch * 512, (nch + 1) * 512)
                            nc.tensor.matmul(
                                ps[:, ccol],
                                atile[:, 2 * kp:2 * kp + 2, mcol],
                                rhs(b, kp)[:, :, ccol],
                                start=(kp == 0),
                                stop=(kp == 1),
                                perf_mode=mybir.MatmulPerfMode.DoubleRow,
                            )
                    # one whole-ic eviction, alternating DVE / ACT
                    if ev % 2 == 0:
                        nc.vector.tensor_scalar_mul(ot[:, ic], ps[:], EVICT)
                    else:
                        nc.scalar.activation(
                            ot[:, ic], ps[:],
                            mybir.ActivationFunctionType.Copy, scale=EVICT)
                    ev += 1
                oview = out_d[b].rearrange("(c p) t f -> p c (t f)", p=128)
                if b < BPC - 1:
                    eng = nc.gpsimd if b % 2 == 0 else nc.scalar
                    eng.dma_start(oview, otiles[b][:])
                else:
                    # last batch: split so the post-matmul tail is short
                    # and rides engines that are idle by then
                    nc.gpsimd.dma_start(oview[:, 0:3], otiles[b][:, 0:3])
                    nc.sync.dma_start(oview[:, 3:4], otiles[b][:, 3:4])

    nc.compile()
    return nc


def prepare(x, adj, alpha, w, d, w2, d2):
    """Host prep: fold parameters, build q. Returns (nc, in_maps)."""
    import ml_dtypes

    x = np.ascontiguousarray(np.asarray(x), np.float32)
    adj = np.asarray(adj)
    alpha = np.asarray(alpha)
    w = np.asarray(w)
    d = np.asarray(d)
    w2 = np.asarray(w2)
    d2 = np.asarray(d2)
    a = 1.0 / (1.0 + np.exp(-alpha.astype(np.float32)))
    A = 0.125 * a[:, None] * adj.astype(np.float32)
    at = np.ascontiguousarray(
        np.clip(A.T * SCALE_AT, -240.0, 240.0)).astype(ml_dtypes.float8_e4m3)

    dc = np.clip(d.astype(np.float32), 0.0, 1.0)
    W = (w.astype(np.float32) * dc) @ w.astype(np.float32).T
    R = W.sum(axis=1)  # [FA]
    d2c = np.clip(d2.astype(np.float32), 0.0, 1.0)
    W2 = (w2.astype(np.float32) * d2c) @ w2.astype(np.float32).T  # [T,T]

    S = x.sum(axis=3)  # [B,N,T]

    # q = 0.5*x + 0.25*(x @_t W2) + 0.25*S*R[:64], kept in host fp32
    q = np.matmul(x.transpose(0, 1, 3, 2), 0.25 * W2).transpose(0, 1, 3, 2)
    q += 0.5 * x
    q += 0.25 * S[..., None] * R[:F]
    xb = x.astype(ml_dtypes.float8_e4m3)

    if "nc" not in _CACHE:
        _CACHE["nc"] = _build()
    nc = _CACHE["nc"]
    in_maps = [
        {"xin": xb[c * BPC:(c + 1) * BPC], "at": at}
        for c in range(N_CORES)
    ]
    _CACHE["q"] = q
    # host-side rank-1 pad columns: relu(0.25 * S * R[64:74])
    _CACHE["pad"] = np.maximum(
        0.25 * S[..., None] * R[F:], 0.0).astype(np.float32)
    return nc, in_maps


def _assemble(results):
    out = np.empty((B, N, T, FA), np.float32)
    adev = np.concatenate(
        [np.asarray(results[c]["out"]) for c in range(N_CORES)], axis=0)
    out[..., :F] = np.maximum(
        _CACHE["q"] + adev.astype(np.float32) * (1.0 / SCALE_A), 0.0)
    out[..., F:] = _CACHE["pad"]
    return out


def kernel(x, adj, alpha, w, d, w2, d2):
    from concourse.bass_utils import run_bass_kernel_spmd

    nc, in_maps = prepare(x, adj, alpha, w, d, w2, d2)
    res = run_bass_kernel_spmd(nc, in_maps, list(range(N_CORES)))
    return _assemble(res.results)


# revision 4
# speedup vs baseline: 1.1541x; 1.1541x over previous
"""Trainium2 Bass kernel for nn_ODEG_8942121911067 (gnn_message_passing).

Math (the reference Euler loop collapses to its last step, f constant):

    out = relu(q + a),  a = 0.125*sigmoid(alpha)_i * (adj @ x_aug)
    q   = 0.5*x_aug + 0.25*S*R + 0.25*(x_aug @_t W2mix)

with x_aug = concat([x, zeros10], -1), S[b,n,t] = sum_f x_aug[b,n,t,f],
R[m] = sum_n ((w*clip(d,0,1)) @ w.T)[m,n], W2mix = (w2*clip(d2,0,1)) @ w2.T.

Device strategy (data-parallel over batch, 4 batches/core on 8 cores).
The device computes the 26 GFLOP adjacency message-passing term
`a = A @ x` in fp8 (A^T pre-scaled 2^20 on host, result scaled 2^13);
the precision-critical linear terms stay in host fp32 and the output is
assembled as relu(q + 2^-13 * a) in numpy.

v1 schedule (coarse DMA, per-batch stores):
  - 6 load dispatches on sync: adj [128,4,512], x b0 split in two
    k-tile halves (so the PE can start after 0.65 MB instead of 1.05),
    then one whole [128,4,1536] tile per remaining batch.
  - 12 HAM-warmup matmuls hold the PE activity window open during the
    load lead-in so the 1.2->2.4 GHz clock ramp happens off the
    critical path.
  - PSUM as [128,1536] 3-bank tiles (pool of 2 + warmup bank): one
    matmul accumulation group per output (b,ic) tile, evicted in ONE
    DVE/ACT scaled-copy per ic (alternating engines so neither gates).
  - Output per batch in one [128,4,1536] SBUF tile; stores are 1
    dispatch each for b0-b2 (alternating gpsimd/scalar) and 2 for b3
    (split so the tail after the last matmul is one 0.59+0.20 MB pair
    on idle engines instead of a serial whole-batch store).
  - HBM traffic/core: 3.15 MB x + 0.26 MB adj in, 3.15 MB a out; the
    96 DoubleRow fp8 matmuls (~216 ns each warm) are the roofline.
"""

import numpy as np

B, N, T, F = 32, 512, 24, 64
NUM_ZEROS = 10
FA = F + NUM_ZEROS  # 74
N_CORES = 8
BPC = B // N_CORES  # batches per core = 4
NT = N // 128  # node chunks = 4
TF = T * F  # 1536
SCALE_AT = 2.0 ** 20  # fp8 subnormal-avoidance scale on the stationary
SCALE_A = 2.0 ** 13  # scale of the returned adjacency term
EVICT = SCALE_A / SCALE_AT  # 2^-7, applied at PSUM eviction

_CACHE = {}


def _build():
    import concourse.mybir as mybir
    import concourse.tile as tile
    from concourse import bacc

    fp8 = mybir.dt.float8e4
    f32 = mybir.dt.float32

    nc = bacc.Bacc("TRN2", target_bir_lowering=False, debug=False,
                   num_devices=N_CORES)
    x_d = nc.dram_tensor("xin", [BPC, N, T, F], fp8, kind="ExternalInput").ap()
    at_d = nc.dram_tensor("at", [N, N], fp8, kind="ExternalInput").ap()
    out_d = nc.dram_tensor("out", [BPC, N, T, F], fp8,
                           kind="ExternalOutput").ap()

    with tile.TileContext(nc) as tc:
        with (
            tc.tile_pool(name="const", bufs=1) as cpool,
            tc.tile_pool(name="xp", bufs=5) as xpool,
            tc.tile_pool(name="op", bufs=4) as opool,
            tc.tile_pool(name="ps", bufs=7, space="PSUM") as pspool,
            tc.tile_pool(name="wp", bufs=1, space="PSUM") as wpool,
        ):
            # ---- loads: all on sync, first-use order, coarse ----
            atile = cpool.tile([128, NT, N], fp8, tag="at")
            nc.sync.dma_start(
                atile[:], at_d[:].rearrange("(c p) n -> p c n", p=128))
            # b0 split into k-tile halves so the first matmul group can
            # start after adj + 0.39 MB instead of adj + 0.79 MB.
            xb0 = []
            for h in range(2):
                xh = xpool.tile([128, 2, TF], fp8, tag="xt",
                                name=f"x0_{h}")
                nc.sync.dma_start(
                    xh[:], x_d[0, h * 256:(h + 1) * 256].rearrange(
                        "(c p) t f -> p c (t f)", p=128))
                xb0.append(xh)
            xts = []
            for b in range(1, BPC):
                xt = xpool.tile([128, NT, TF], fp8, tag="xt",
                                name=f"x{b}")
                nc.sync.dma_start(
                    xt[:], x_d[b].rearrange("(c p) t f -> p c (t f)", p=128))
                xts.append(xt)

            def rhs(b, kp):
                if b == 0:
                    return xb0[kp][:]
                return xts[b - 1][:, 2 * kp:2 * kp + 2]

            # ---- HAM warmup: hold the PE activity window open ----
            wmov = cpool.tile([128, 512], fp8, tag="wmov")
            nc.vector.memset(wmov[:], 0)
            wps = wpool.tile([128, 512], f32, tag="wps", name="wps")
            for _ in range(12):
                nc.tensor.matmul(wps[:], wmov[:, 0:128], wmov[:],
                                 start=True, stop=True)

            # ---- main stream ----
            ev = 0
            NCH = TF // 512  # 3 psum banks per (b, ic) accumulation group
            otiles = []
            for b in range(BPC):
                ot = opool.tile([128, NT, TF], fp8, tag="ot", name=f"o{b}")
                otiles.append(ot)
                for ic in range(NT):
                    mcol = slice(ic * 128, (ic + 1) * 128)
                    # one PSUM bank per 512-col chunk: banks free as soon
                    # as their chunk evicts, so the PE never waits on a
                    # whole-group eviction
                    pss = [pspool.tile([128, 512], f32, tag="ps",
                                       name=f"ps_{b}_{ic}_{j}")
                           for j in range(NCH)]
                    for kp in range(2):
                        for nch in range(NCH):
                            ccol = slice(nch * 512, (nch + 1) * 512)
                            nc.tensor.matmul(
                                pss[nch][:],
                                atile[:, 2 * kp:2 * kp + 2, mcol],
                                rhs(b, kp)[:, :, ccol],
                                start=(kp == 0),
                                stop=(kp == 1),
                                perf_mode=mybir.MatmulPerfMode.DoubleRow,
                            )
                    # per-chunk evictions, alternating DVE / ACT
                    for nch in range(NCH):
                        ccol = slice(nch * 512, (nch + 1) * 512)
                        if ev % 2 == 0:
                            nc.vector.tensor_scalar_mul(
                                ot[:, ic, ccol], pss[nch][:], EVICT)
                        else:
                            nc.scalar.activation(
                                ot[:, ic, ccol], pss[nch][:],
                                mybir.ActivationFunctionType.Copy,
                                scale=EVICT)
                        ev += 1
                oview = out_d[b].rearrange("(c p) t f -> p c (t f)", p=128)
                if b < BPC - 1:
                    eng = nc.gpsimd if b % 2 == 0 else nc.scalar
                    eng.dma_start(oview, otiles[b][:])
                else:
                    # last batch: split so the post-matmul tail is short
                    # and rides engines that are idle by then
                    nc.gpsimd.dma_start(oview[:, 0:3], otiles[b][:, 0:3])
                    nc.sync.dma_start(oview[:, 3:4], otiles[b][:, 3:4])

    nc.compile()
    return nc


def prepare(x, adj, alpha, w, d, w2, d2):
    """Host prep: fold parameters, build q. Returns (nc, in_maps)."""
    import ml_dtypes

    x = np.ascontiguousarray(np.asarray(x), np.float32)
    adj = np.asarray(adj)
    alpha = np.asarray(alpha)
    w = np.asarray(w)
    d = np.asarray(d)
    w2 = np.asarray(w2)
    d2 = np.asarray(d2)
    a = 1.0 / (1.0 + np.exp(-alpha.astype(np.float32)))
    A = 0.125 * a[:, None] * adj.astype(np.float32)
    at = np.ascontiguousarray(
        np.clip(A.T * SCALE_AT, -240.0, 240.0)).astype(ml_dtypes.float8_e4m3)

    dc = np.clip(d.astype(np.float32), 0.0, 1.0)
    W = (w.astype(np.float32) * dc) @ w.astype(np.float32).T
    R = W.sum(axis=1)  # [FA]
    d2c = np.clip(d2.astype(np.float32), 0.0, 1.0)
    W2 = (w2.astype(np.float32) * d2c) @ w2.astype(np.float32).T  # [T,T]

    S = x.sum(axis=3)  # [B,N,T]

    # q = 0.5*x + 0.25*(x @_t W2) + 0.25*S*R[:64], kept in host fp32
    q = np.matmul(x.transpose(0, 1, 3, 2), 0.25 * W2).transpose(0, 1, 3, 2)
    q += 0.5 * x
    q += 0.25 * S[..., None] * R[:F]
    xb = x.astype(ml_dtypes.float8_e4m3)

    if "nc" not in _CACHE:
        _CACHE["nc"] = _build()
    nc = _CACHE["nc"]
    in_maps = [
        {"xin": xb[c * BPC:(c + 1) * BPC], "at": at}
        for c in range(N_CORES)
    ]
    _CACHE["q"] = q
    # host-side rank-1 pad columns: relu(0.25 * S * R[64:74])
    _CACHE["pad"] = np.maximum(
        0.25 * S[..., None] * R[F:], 0.0).astype(np.float32)
    return nc, in_maps


def _assemble(results):
    out = np.empty((B, N, T, FA), np.float32)
    adev = np.concatenate(
        [np.asarray(results[c]["out"]) for c in range(N_CORES)], axis=0)
    out[..., :F] = np.maximum(
        _CACHE["q"] + adev.astype(np.float32) * (1.0 / SCALE_A), 0.0)
    out[..., F:] = _CACHE["pad"]
    return out


def kernel(x, adj, alpha, w, d, w2, d2):
    from concourse.bass_utils import run_bass_kernel_spmd

    nc, in_maps = prepare(x, adj, alpha, w, d, w2, d2)
    res = run_bass_kernel_spmd(nc, in_maps, list(range(N_CORES)))
    return _assemble(res.results)


# revision 6
# speedup vs baseline: 1.1980x; 1.0380x over previous
"""Trainium2 Bass kernel for nn_ODEG_8942121911067 (gnn_message_passing).

Math (the reference Euler loop collapses to its last step, f constant):

    out = relu(q + a),  a = 0.125*sigmoid(alpha)_i * (adj @ x_aug)
    q   = 0.5*x_aug + 0.25*S*R + 0.25*(x_aug @_t W2mix)

with x_aug = concat([x, zeros10], -1), S[b,n,t] = sum_f x_aug[b,n,t,f],
R[m] = sum_n ((w*clip(d,0,1)) @ w.T)[m,n], W2mix = (w2*clip(d2,0,1)) @ w2.T.

Device strategy (data-parallel over batch, 4 batches/core on 8 cores).
The device computes the 26 GFLOP adjacency message-passing term
`a = A @ x` in fp8 (A^T pre-scaled 2^20 on host since raw A values are
fp8-subnormal; result scaled 2^13); the precision-critical linear terms
stay in host fp32 and the output is assembled as relu(q + 2^-13 * a).

RAW BASS (no TileContext): the Tile framework's prologue/epilogue
(pool barriers + ~300-instruction semaphore teardown) costs ~10 us of
the measured window regardless of kernel content. This kernel
hand-schedules five engine queues with 5 semaphores instead:

  sync   : 6 load dispatches (adj, x b0 in two k-halves so the PE can
           start after 0.65 MB, then one tile per batch), then the
           final small store of b3/ic3.
  tensor : 8 HAM-warmup matmuls (hold the clock-ramp window open during
           the load lead-in), then 96 DoubleRow fp8 matmuls, gap-free;
           waits: s_ld per batch, s_evv/s_evs for PSUM bank reuse
           (6 rotating banks, freed per 512-col chunk eviction).
  vector : even-chunk PSUM evictions (scaled copy to fp8).
  scalar : odd-chunk evictions + the b1 store dispatch.
  gpsimd : b0/b2/b3[ic0:3] store dispatches, final wait for all store
           completions, then the semaphore clear for re-execution
           safety and one all-engine barrier.

HBM traffic/core: 3.15 MB x + 0.26 MB adj in, 3.15 MB a out; the 96
matmuls (~216 ns each warm) are the roofline.
"""

import numpy as np

B, N, T, F = 32, 512, 24, 64
NUM_ZEROS = 10
FA = F + NUM_ZEROS  # 74
N_CORES = 8
BPC = B // N_CORES  # batches per core = 4
NT = N // 128  # node chunks = 4
TF = T * F  # 1536
NCH = TF // 512  # 512-col chunks per (b, ic) group = 3
NBANK = 6  # rotating PSUM banks for the matmul stream
SCALE_AT = 2.0 ** 20  # fp8 subnormal-avoidance scale on the stationary
SCALE_A = 2.0 ** 13  # scale of the returned adjacency term
EVICT = SCALE_A / SCALE_AT  # 2^-7, applied at PSUM eviction

_CACHE = {}


def _build():
    import concourse.mybir as mybir
    from concourse import bacc

    fp8 = mybir.dt.float8e4
    f32 = mybir.dt.float32
    DR = mybir.MatmulPerfMode.DoubleRow

    nc = bacc.Bacc("TRN2", target_bir_lowering=False, debug=False,
                   num_devices=N_CORES)
    x_d = nc.dram_tensor("xin", [BPC, N, T, F], fp8, kind="ExternalInput").ap()
    at_d = nc.dram_tensor("at", [N, N], fp8, kind="ExternalInput").ap()
    out_d = nc.dram_tensor("out", [BPC, N, T, F], fp8,
                           kind="ExternalOutput").ap()

    # ---- on-chip buffers ----
    atile = nc.alloc_sbuf_tensor("atile", [128, NT, N], fp8).ap()
    x00 = nc.alloc_sbuf_tensor("x00", [128, 2, TF], fp8).ap()
    x01 = nc.alloc_sbuf_tensor("x01", [128, 2, TF], fp8).ap()
    xts = [nc.alloc_sbuf_tensor(f"x{b}", [128, NT, TF], fp8).ap()
           for b in range(1, BPC)]
    ots = [nc.alloc_sbuf_tensor(f"o{b}", [128, NT, TF], fp8).ap()
           for b in range(BPC)]
    wmov = nc.alloc_sbuf_tensor("wmov", [128, 512], fp8).ap()
    banks = [nc.alloc_psum_tensor(f"pb{j}", [128, 512], f32).ap()
             for j in range(NBANK)]
    wps = nc.alloc_psum_tensor("wps", [128, 512], f32).ap()

    # One semaphore per load DMA: a shared counting sem is only sound at
    # its FULL count (per-ring completion order does not bound partial
    # counts), and the PE waits at per-load thresholds.
    s_lds = [nc.alloc_semaphore(f"s_ld{i}") for i in range(6)]
    s_mm = nc.alloc_semaphore("s_mm")
    s_evv = nc.alloc_semaphore("s_evv")
    s_evs = nc.alloc_semaphore("s_evs")
    s_st = nc.alloc_semaphore("s_st")  # full-count wait only (5*16)

    # ---- sync: loads in first-use order, then all 5 stores ----
    nc.sync.dma_start(
        atile[:], at_d[:].rearrange("(c p) n -> p c n", p=128)
    ).then_inc(s_lds[0], 16)
    for i, (h, xt) in enumerate(((0, x00), (1, x01))):
        nc.sync.dma_start(
            xt[:], x_d[0, h * 256:(h + 1) * 256].rearrange(
                "(c p) t f -> p c (t f)", p=128)
        ).then_inc(s_lds[1 + h], 16)
    for b in range(1, BPC):
        nc.sync.dma_start(
            xts[b - 1][:],
            x_d[b].rearrange("(c p) t f -> p c (t f)", p=128)
        ).then_inc(s_lds[2 + b], 16)

    def rhs(b, kp):
        if b == 0:
            return (x00, x01)[kp]
        return xts[b - 1][:, 2 * kp:2 * kp + 2]

    # ---- tensor: warmups then the 96-matmul stream ----
    for _ in range(5):
        nc.tensor.matmul(wps[:], wmov[:, 0:128], wmov[:],
                         start=True, stop=True)
    for b in range(BPC):
        for ic in range(NT):
            mcol = slice(ic * 128, (ic + 1) * 128)
            g0 = (b * NT + ic) * NCH
            for kp in range(2):
                for nch in range(NCH):
                    g = g0 + nch
                    if kp == 0:
                        if ic == 0 and nch == 0:
                            if b == 0:
                                nc.tensor.wait_ge(s_lds[0], 16)  # adj
                                nc.tensor.wait_ge(s_lds[1], 16)  # x0 kp0
                            else:
                                nc.tensor.wait_ge(s_lds[2 + b], 16)
                        if g >= NBANK:
                            f_ = g - NBANK  # chunk that frees this bank
                            if f_ % 2 == 0:
                                nc.tensor.wait_ge(s_evv, f_ // 2 + 1)
                            else:
                                nc.tensor.wait_ge(s_evs, (f_ + 1) // 2)
                    elif b == 0 and ic == 0 and nch == 0:
                        nc.tensor.wait_ge(s_lds[2], 16)  # x0 kp1 half
                    mm = nc.tensor.matmul(
                        banks[g % NBANK][:],
                        atile[:, 2 * kp:2 * kp + 2, mcol],
                        rhs(b, kp)[:, :, nch * 512:(nch + 1) * 512],
                        start=(kp == 0), stop=(kp == 1), perf_mode=DR)
                    if kp == 1:
                        mm.then_inc(s_mm, 1)

    # ---- evictions: vector = even chunks, scalar = odd chunks ----
    for g in range(BPC * NT * NCH):
        b, r = divmod(g, NT * NCH)
        ic, nch = divmod(r, NCH)
        dst = ots[b][:, ic, nch * 512:(nch + 1) * 512]
        src = banks[g % NBANK][:]
        if g % 2 == 0:
            nc.vector.wait_ge(s_mm, g + 1)
            nc.vector.tensor_scalar_mul(dst, src, EVICT).then_inc(s_evv, 1)
        else:
            nc.scalar.wait_ge(s_mm, g + 1)
            nc.scalar.activation(
                dst, src, mybir.ActivationFunctionType.Copy, scale=EVICT
            ).then_inc(s_evs, 1)

    def oview(b):
        return out_d[b].rearrange("(c p) t f -> p c (t f)", p=128)

    # ---- sync: stores (HWDGE: completion sems post fast), in readiness
    # order; b3 split in halves so the post-stream tail is short ----
    for evv, evs, dst, srcv in (
        (6, 6, oview(0), ots[0][:]),
        (12, 12, oview(1), ots[1][:]),
        (18, 18, oview(2), ots[2][:]),
        (21, 21, oview(3)[:, 0:2], ots[3][:, 0:2]),
        (24, 24, oview(3)[:, 2:4], ots[3][:, 2:4]),
    ):
        nc.sync.wait_ge(s_evv, evv)
        nc.sync.wait_ge(s_evs, evs)
        nc.sync.dma_start(dst, srcv).then_inc(s_st, 16)

    # ---- end: prove stores done, clear sems for re-execution safety ----
    nc.gpsimd.wait_ge(s_st, 5 * 16)
    nc.clear_and_free_semaphores(s_lds + [s_mm, s_evv, s_evs, s_st])
    nc.all_engine_barrier()

    nc.compile()
    return nc


def prepare(x, adj, alpha, w, d, w2, d2):
    """Host prep: fold parameters, build q. Returns (nc, in_maps)."""
    import ml_dtypes

    x = np.ascontiguousarray(np.asarray(x), np.float32)
    adj = np.asarray(adj)
    alpha = np.asarray(alpha)
    w = np.asarray(w)
    d = np.asarray(d)
    w2 = np.asarray(w2)
    d2 = np.asarray(d2)
    a = 1.0 / (1.0 + np.exp(-alpha.astype(np.float32)))
    A = 0.125 * a[:, None] * adj.astype(np.float32)
    at = np.ascontiguousarray(
        np.clip(A.T * SCALE_AT, -240.0, 240.0)).astype(ml_dtypes.float8_e4m3)

    dc = np.clip(d.astype(np.float32), 0.0, 1.0)
    W = (w.astype(np.float32) * dc) @ w.astype(np.float32).T
    R = W.sum(axis=1)  # [FA]
    d2c = np.clip(d2.astype(np.float32), 0.0, 1.0)
    W2 = (w2.astype(np.float32) * d2c) @ w2.astype(np.float32).T  # [T,T]

    S = x.sum(axis=3)  # [B,N,T]

    # q = 0.5*x + 0.25*(x @_t W2) + 0.25*S*R[:64], kept in host fp32
    q = np.matmul(x.transpose(0, 1, 3, 2), 0.25 * W2).transpose(0, 1, 3, 2)
    q += 0.5 * x
    q += 0.25 * S[..., None] * R[:F]
    xb = x.astype(ml_dtypes.float8_e4m3)

    if "nc" not in _CACHE:
        _CACHE["nc"] = _build()
    nc = _CACHE["nc"]
    in_maps = [
        {"xin": xb[c * BPC:(c + 1) * BPC], "at": at}
        for c in range(N_CORES)
    ]
    _CACHE["q"] = q
    # host-side rank-1 pad columns: relu(0.25 * S * R[64:74])
    _CACHE["pad"] = np.maximum(
        0.25 * S[..., None] * R[F:], 0.0).astype(np.float32)
    return nc, in_maps


def _assemble(results):
    out = np.empty((B, N, T, FA), np.float32)
    adev = np.concatenate(
        [np.asarray(results[c]["out"]) for c in range(N_CORES)], axis=0)
    out[..., :F] = np.maximum(
        _CACHE["q"] + adev.astype(np.float32) * (1.0 / SCALE_A), 0.0)
    out[..., F:] = _CACHE["pad"]
    return out


def kernel(x, adj, alpha, w, d, w2, d2):
    from concourse.bass_utils import run_bass_kernel_spmd

    nc, in_maps = prepare(x, adj, alpha, w, d, w2, d2)
    res = run_bass_kernel_spmd(nc, in_maps, list(range(N_CORES)))
    return _assemble(res.results)


# revision 8
# speedup vs baseline: 1.2675x; 1.0581x over previous
"""Trainium2 Bass kernel for nn_ODEG_8942121911067 (gnn_message_passing).

Math (the reference Euler loop collapses to its last step, f constant):

    out = relu(q + a),  a = 0.125*sigmoid(alpha)_i * (adj @ x_aug)
    q   = 0.5*x_aug + 0.25*S*R + 0.25*(x_aug @_t W2mix)

with x_aug = concat([x, zeros10], -1), S[b,n,t] = sum_f x_aug[b,n,t,f],
R[m] = sum_n ((w*clip(d,0,1)) @ w.T)[m,n], W2mix = (w2*clip(d2,0,1)) @ w2.T.

Device strategy (data-parallel over batch, 4 batches/core on 8 cores).
The device computes the 26 GFLOP adjacency message-passing term
`a = A @ x` in fp8 (A^T pre-scaled 2^20 on host since raw A values are
fp8-subnormal; result scaled 2^13); the precision-critical linear terms
stay in host fp32 and the output is assembled as relu(q + 2^-13 * a).

RAW BASS (no TileContext): the Tile framework's prologue/epilogue
(pool barriers + ~300-instruction semaphore teardown) costs ~10 us of
the measured window regardless of kernel content. This kernel
hand-schedules five engine queues with 5 semaphores instead:

  sync   : 6 load dispatches (adj, x b0 in two k-halves so the PE can
           start after 0.65 MB, then one tile per batch), then the
           final small store of b3/ic3.
  tensor : 8 HAM-warmup matmuls (hold the clock-ramp window open during
           the load lead-in), then 96 DoubleRow fp8 matmuls, gap-free;
           waits: s_ld per batch, s_evv/s_evs for PSUM bank reuse
           (6 rotating banks, freed per 512-col chunk eviction).
  vector : even-chunk PSUM evictions (scaled copy to fp8).
  scalar : odd-chunk evictions + the b1 store dispatch.
  gpsimd : b0/b2/b3[ic0:3] store dispatches, final wait for all store
           completions, then the semaphore clear for re-execution
           safety and one all-engine barrier.

HBM traffic/core: 3.15 MB x + 0.26 MB adj in, 3.15 MB a out; the 96
matmuls (~216 ns each warm) are the roofline.
"""

import numpy as np

B, N, T, F = 32, 512, 24, 64
NUM_ZEROS = 10
FA = F + NUM_ZEROS  # 74
N_CORES = 8
BPC = B // N_CORES  # batches per core = 4
NT = N // 128  # node chunks = 4
TF = T * F  # 1536
NCH = TF // 512  # 512-col chunks per (b, ic) group = 3
NBANK = 6  # rotating PSUM banks for the matmul stream
SCALE_AT = 2.0 ** 20  # fp8 subnormal-avoidance scale on the stationary
SCALE_A = 2.0 ** 13  # scale of the returned adjacency term
EVICT = SCALE_A / SCALE_AT  # 2^-7, applied at PSUM eviction

_CACHE = {}


def _build():
    import concourse.mybir as mybir
    from concourse import bacc

    fp8 = mybir.dt.float8e4
    f32 = mybir.dt.float32
    DR = mybir.MatmulPerfMode.DoubleRow

    nc = bacc.Bacc("TRN2", target_bir_lowering=False, debug=False,
                   num_devices=N_CORES)
    x_d = nc.dram_tensor("xin", [BPC, N, T, F], fp8, kind="ExternalInput").ap()
    at_d = nc.dram_tensor("at", [N, N], fp8, kind="ExternalInput").ap()
    out_d = nc.dram_tensor("out", [BPC, N, T, F], fp8,
                           kind="ExternalOutput").ap()

    # ---- on-chip buffers ----
    atile = nc.alloc_sbuf_tensor("atile", [128, NT, N], fp8).ap()
    x00 = nc.alloc_sbuf_tensor("x00", [128, 2, TF], fp8).ap()
    x01 = nc.alloc_sbuf_tensor("x01", [128, 2, TF], fp8).ap()
    xts = [nc.alloc_sbuf_tensor(f"x{b}", [128, NT, TF], fp8).ap()
           for b in range(1, BPC)]
    ots = [nc.alloc_sbuf_tensor(f"o{b}", [128, NT, TF], fp8).ap()
           for b in range(BPC)]
    wmov = nc.alloc_sbuf_tensor("wmov", [128, 512], fp8).ap()
    banks = [nc.alloc_psum_tensor(f"pb{j}", [128, 512], f32).ap()
             for j in range(NBANK)]
    wps = nc.alloc_psum_tensor("wps", [128, 512], f32).ap()

    # One semaphore per load DMA: a shared counting sem is only sound at
    # its FULL count (per-ring completion order does not bound partial
    # counts), and the PE waits at per-load thresholds.
    s_lds = [nc.alloc_semaphore(f"s_ld{i}") for i in range(6)]
    s_mm = nc.alloc_semaphore("s_mm")
    s_evv = nc.alloc_semaphore("s_evv")
    s_evs = nc.alloc_semaphore("s_evs")
    # stores must carry a sem update (walrus codegen requires one), but
    # nothing waits on it and it is never cleared: the NEFF epilogue's
    # ring drain covers store completion, overlapped with the last wire
    s_st = nc.alloc_semaphore("s_st")

    # ---- loads: adj on scalar (parallel with x00's dispatch on sync),
    # x tiles on sync in first-use order ----
    nc.scalar.dma_start(
        atile[:], at_d[:].rearrange("(c p) n -> p c n", p=128)
    ).then_inc(s_lds[0], 16)
    for i, (h, xt) in enumerate(((0, x00), (1, x01))):
        nc.sync.dma_start(
            xt[:], x_d[0, h * 256:(h + 1) * 256].rearrange(
                "(c p) t f -> p c (t f)", p=128)
        ).then_inc(s_lds[1 + h], 16)
    for b in range(1, BPC):
        nc.sync.dma_start(
            xts[b - 1][:],
            x_d[b].rearrange("(c p) t f -> p c (t f)", p=128)
        ).then_inc(s_lds[2 + b], 16)

    def rhs(b, kp):
        if b == 0:
            return (x00, x01)[kp]
        return xts[b - 1][:, 2 * kp:2 * kp + 2]

    # ---- tensor: warmups then the 96-matmul stream ----
    for _ in range(3):
        nc.tensor.matmul(wps[:], wmov[:, 0:128], wmov[:],
                         start=True, stop=True)
    for b in range(BPC):
        for ic in range(NT):
            mcol = slice(ic * 128, (ic + 1) * 128)
            g0 = (b * NT + ic) * NCH
            for kp in range(2):
                for nch in range(NCH):
                    g = g0 + nch
                    if kp == 0:
                        if ic == 0 and nch == 0:
                            if b == 0:
                                nc.tensor.wait_ge(s_lds[0], 16)  # adj
                                nc.tensor.wait_ge(s_lds[1], 16)  # x0 kp0
                            else:
                                nc.tensor.wait_ge(s_lds[2 + b], 16)
                        if g >= NBANK:
                            f_ = g - NBANK  # chunk that frees this bank
                            if f_ % 2 == 0:
                                nc.tensor.wait_ge(s_evv, f_ // 2 + 1)
                            else:
                                nc.tensor.wait_ge(s_evs, (f_ + 1) // 2)
                    elif b == 0 and ic == 0 and nch == 0:
                        nc.tensor.wait_ge(s_lds[2], 16)  # x0 kp1 half
                    mm = nc.tensor.matmul(
                        banks[g % NBANK][:],
                        atile[:, 2 * kp:2 * kp + 2, mcol],
                        rhs(b, kp)[:, :, nch * 512:(nch + 1) * 512],
                        start=(kp == 0), stop=(kp == 1), perf_mode=DR)
                    if kp == 1:
                        mm.then_inc(s_mm, 1)

    # ---- evictions: vector = even chunks, scalar = odd chunks ----
    for g in range(BPC * NT * NCH):
        b, r = divmod(g, NT * NCH)
        ic, nch = divmod(r, NCH)
        dst = ots[b][:, ic, nch * 512:(nch + 1) * 512]
        src = banks[g % NBANK][:]
        if g % 2 == 0:
            nc.vector.wait_ge(s_mm, g + 1)
            nc.vector.tensor_scalar_mul(dst, src, EVICT).then_inc(s_evv, 1)
        else:
            nc.scalar.wait_ge(s_mm, g + 1)
            nc.scalar.activation(
                dst, src, mybir.ActivationFunctionType.Copy, scale=EVICT
            ).then_inc(s_evs, 1)

    def oview(b):
        return out_d[b].rearrange("(c p) t f -> p c (t f)", p=128)

    # ---- sync: stores (HWDGE: completion sems post fast), in readiness
    # order; b3 split in halves so the post-stream tail is short ----
    for evv, evs, dst, srcv in (
        (6, 6, oview(0), ots[0][:]),
        (12, 12, oview(1), ots[1][:]),
        (18, 18, oview(2), ots[2][:]),
        (21, 21, oview(3)[:, 0:2], ots[3][:, 0:2]),
        (24, 24, oview(3)[:, 2:4], ots[3][:, 2:4]),
    ):
        nc.sync.wait_ge(s_evv, evv)
        nc.sync.wait_ge(s_evs, evs)
        nc.sync.dma_start(dst, srcv).then_inc(s_st, 16)

    # ---- end: clear sems for re-execution safety once all eviction /
    # matmul / load sem traffic has retired (evv/evs full counts imply
    # everything upstream). Store DMAs carry no sem: the NEFF epilogue's
    # own ring drain covers them, overlapped with the final wire. ----
    nc.gpsimd.wait_ge(s_evv, 24)
    nc.gpsimd.wait_ge(s_evs, 24)
    nc.clear_and_free_semaphores(s_lds + [s_mm, s_evv, s_evs])
    nc.all_engine_barrier()

    nc.compile()
    return nc


def prepare(x, adj, alpha, w, d, w2, d2):
    """Host prep: fold parameters, build q. Returns (nc, in_maps)."""
    import ml_dtypes

    x = np.ascontiguousarray(np.asarray(x), np.float32)
    adj = np.asarray(adj)
    alpha = np.asarray(alpha)
    w = np.asarray(w)
    d = np.asarray(d)
    w2 = np.asarray(w2)
    d2 = np.asarray(d2)
    a = 1.0 / (1.0 + np.exp(-alpha.astype(np.float32)))
    A = 0.125 * a[:, None] * adj.astype(np.float32)
    at = np.ascontiguousarray(
        np.clip(A.T * SCALE_AT, -240.0, 240.0)).astype(ml_dtypes.float8_e4m3)

    dc = np.clip(d.astype(np.float32), 0.0, 1.0)
    W = (w.astype(np.float32) * dc) @ w.astype(np.float32).T
    R = W.sum(axis=1)  # [FA]
    d2c = np.clip(d2.astype(np.float32), 0.0, 1.0)
    W2 = (w2.astype(np.float32) * d2c) @ w2.astype(np.float32).T  # [T,T]

    S = x.sum(axis=3)  # [B,N,T]

    # q = 0.5*x + 0.25*(x @_t W2) + 0.25*S*R[:64], kept in host fp32
    q = np.matmul(x.transpose(0, 1, 3, 2), 0.25 * W2).transpose(0, 1, 3, 2)
    q += 0.5 * x
    q += 0.25 * S[..., None] * R[:F]
    xb = x.astype(ml_dtypes.float8_e4m3)

    if "nc" not in _CACHE:
        _CACHE["nc"] = _build()
    nc = _CACHE["nc"]
    in_maps = [
        {"xin": xb[c * BPC:(c + 1) * BPC], "at": at}
        for c in range(N_CORES)
    ]
    _CACHE["q"] = q
    # host-side rank-1 pad columns: relu(0.25 * S * R[64:74])
    _CACHE["pad"] = np.maximum(
        0.25 * S[..., None] * R[F:], 0.0).astype(np.float32)
    return nc, in_maps


def _assemble(results):
    out = np.empty((B, N, T, FA), np.float32)
    adev = np.concatenate(
        [np.asarray(results[c]["out"]) for c in range(N_CORES)], axis=0)
    out[..., :F] = np.maximum(
        _CACHE["q"] + adev.astype(np.float32) * (1.0 / SCALE_A), 0.0)
    out[..., F:] = _CACHE["pad"]
    return out


def kernel(x, adj, alpha, w, d, w2, d2):
    from concourse.bass_utils import run_bass_kernel_spmd

    nc, in_maps = prepare(x, adj, alpha, w, d, w2, d2)
    res = run_bass_kernel_spmd(nc, in_maps, list(range(N_CORES)))
    return _assemble(res.results)


# revision 9
# speedup vs baseline: 1.3163x; 1.0385x over previous
"""Trainium2 Bass kernel for nn_ODEG_8942121911067 (gnn_message_passing).

Math (the reference Euler loop collapses to its last step, f constant):

    out = relu(q + a),  a = 0.125*sigmoid(alpha)_i * (adj @ x_aug)
    q   = 0.5*x_aug + 0.25*S*R + 0.25*(x_aug @_t W2mix)

with x_aug = concat([x, zeros10], -1), S[b,n,t] = sum_f x_aug[b,n,t,f],
R[m] = sum_n ((w*clip(d,0,1)) @ w.T)[m,n], W2mix = (w2*clip(d2,0,1)) @ w2.T.

Device strategy (data-parallel over batch, 4 batches/core on 8 cores).
The device computes the 26 GFLOP adjacency message-passing term
`a = A @ x` in fp8 (A^T pre-scaled 2^20 on host since raw A values are
fp8-subnormal; result scaled 2^13); the precision-critical linear terms
stay in host fp32 and the output is assembled as relu(q + 2^-13 * a).

RAW BASS (no TileContext): the Tile framework's prologue/epilogue
(pool barriers + ~300-instruction semaphore teardown) costs ~10 us of
the measured window regardless of kernel content. This kernel
hand-schedules five engine queues with 5 semaphores instead:

  sync   : 6 load dispatches (adj, x b0 in two k-halves so the PE can
           start after 0.65 MB, then one tile per batch), then the
           final small store of b3/ic3.
  tensor : 8 HAM-warmup matmuls (hold the clock-ramp window open during
           the load lead-in), then 96 DoubleRow fp8 matmuls, gap-free;
           waits: s_ld per batch, s_evv/s_evs for PSUM bank reuse
           (6 rotating banks, freed per 512-col chunk eviction).
  vector : even-chunk PSUM evictions (scaled copy to fp8).
  scalar : odd-chunk evictions + the b1 store dispatch.
  gpsimd : b0/b2/b3[ic0:3] store dispatches, final wait for all store
           completions, then the semaphore clear for re-execution
           safety and one all-engine barrier.

HBM traffic/core: 3.15 MB x + 0.26 MB adj in, 3.15 MB a out; the 96
matmuls (~216 ns each warm) are the roofline.
"""

import numpy as np

B, N, T, F = 32, 512, 24, 64
NUM_ZEROS = 10
FA = F + NUM_ZEROS  # 74
N_CORES = 8
BPC = B // N_CORES  # batches per core = 4
NT = N // 128  # node chunks = 4
TF = T * F  # 1536
NCH = TF // 512  # 512-col chunks per (b, ic) group = 3
NBANK = 6  # rotating PSUM banks for the matmul stream
SCALE_AT = 2.0 ** 20  # fp8 subnormal-avoidance scale on the stationary
SCALE_A = 2.0 ** 13  # scale of the returned adjacency term
EVICT = SCALE_A / SCALE_AT  # 2^-7, applied at PSUM eviction

_CACHE = {}


def _build():
    import concourse.mybir as mybir
    from concourse import bacc

    fp8 = mybir.dt.float8e4
    f32 = mybir.dt.float32
    DR = mybir.MatmulPerfMode.DoubleRow

    nc = bacc.Bacc("TRN2", target_bir_lowering=False, debug=False,
                   num_devices=N_CORES)
    x_d = nc.dram_tensor("xin", [BPC, N, T, F], fp8, kind="ExternalInput").ap()
    at_d = nc.dram_tensor("at", [N, N], fp8, kind="ExternalInput").ap()
    out_d = nc.dram_tensor("out", [BPC, N, T, F], fp8,
                           kind="ExternalOutput").ap()

    # ---- on-chip buffers ----
    atile = nc.alloc_sbuf_tensor("atile", [128, NT, N], fp8).ap()
    x00 = nc.alloc_sbuf_tensor("x00", [128, 2, TF], fp8).ap()
    x01 = nc.alloc_sbuf_tensor("x01", [128, 2, TF], fp8).ap()
    xts = [nc.alloc_sbuf_tensor(f"x{b}", [128, NT, TF], fp8).ap()
           for b in range(1, BPC)]
    ots = [nc.alloc_sbuf_tensor(f"o{b}", [128, NT, TF], fp8).ap()
           for b in range(BPC)]
    wmov = nc.alloc_sbuf_tensor("wmov", [128, 512], fp8).ap()
    banks = [nc.alloc_psum_tensor(f"pb{j}", [128, 512], f32).ap()
             for j in range(NBANK)]
    wps = nc.alloc_psum_tensor("wps", [128, 512], f32).ap()

    # One semaphore per load DMA: a shared counting sem is only sound at
    # its FULL count (per-ring completion order does not bound partial
    # counts), and the PE waits at per-load thresholds.
    s_lds = [nc.alloc_semaphore(f"s_ld{i}") for i in range(6)]
    s_mm = nc.alloc_semaphore("s_mm")
    s_evv = nc.alloc_semaphore("s_evv")
    s_evs = nc.alloc_semaphore("s_evs")
    # stores must carry a sem update (walrus codegen requires one), but
    # nothing waits on it and it is never cleared: the NEFF epilogue's
    # ring drain covers store completion, overlapped with the last wire
    s_st = nc.alloc_semaphore("s_st")

    # ---- loads: adj on scalar (parallel with x00's dispatch on sync),
    # x tiles on sync in first-use order ----
    nc.scalar.dma_start(
        atile[:], at_d[:].rearrange("(c p) n -> p c n", p=128)
    ).then_inc(s_lds[0], 16)
    for i, (h, xt) in enumerate(((0, x00), (1, x01))):
        nc.sync.dma_start(
            xt[:], x_d[0, h * 256:(h + 1) * 256].rearrange(
                "(c p) t f -> p c (t f)", p=128)
        ).then_inc(s_lds[1 + h], 16)
    for b in range(1, BPC):
        nc.sync.dma_start(
            xts[b - 1][:],
            x_d[b].rearrange("(c p) t f -> p c (t f)", p=128)
        ).then_inc(s_lds[2 + b], 16)

    def rhs(b, kp):
        if b == 0:
            return (x00, x01)[kp]
        return xts[b - 1][:, 2 * kp:2 * kp + 2]

    # ---- tensor: warmups then the 96-matmul stream ----
    for _ in range(10):
        nc.tensor.matmul(wps[:], wmov[:, 0:128], wmov[:],
                         start=True, stop=True)
    for b in range(BPC):
        for ic in range(NT):
            mcol = slice(ic * 128, (ic + 1) * 128)
            g0 = (b * NT + ic) * NCH
            for kp in range(2):
                for nch in range(NCH):
                    g = g0 + nch
                    if kp == 0:
                        if ic == 0 and nch == 0:
                            if b == 0:
                                nc.tensor.wait_ge(s_lds[0], 16)  # adj
                                nc.tensor.wait_ge(s_lds[1], 16)  # x0 kp0
                            else:
                                nc.tensor.wait_ge(s_lds[2 + b], 16)
                        if g >= NBANK:
                            f_ = g - NBANK  # chunk that frees this bank
                            if f_ % 2 == 0:
                                nc.tensor.wait_ge(s_evv, f_ // 2 + 1)
                            else:
                                nc.tensor.wait_ge(s_evs, (f_ + 1) // 2)
                    elif b == 0 and ic == 0 and nch == 0:
                        nc.tensor.wait_ge(s_lds[2], 16)  # x0 kp1 half
                    mm = nc.tensor.matmul(
                        banks[g % NBANK][:],
                        atile[:, 2 * kp:2 * kp + 2, mcol],
                        rhs(b, kp)[:, :, nch * 512:(nch + 1) * 512],
                        start=(kp == 0), stop=(kp == 1), perf_mode=DR)
                    if kp == 1:
                        mm.then_inc(s_mm, 1)

    # ---- evictions: vector = even chunks, scalar = odd chunks ----
    for g in range(BPC * NT * NCH):
        b, r = divmod(g, NT * NCH)
        ic, nch = divmod(r, NCH)
        dst = ots[b][:, ic, nch * 512:(nch + 1) * 512]
        src = banks[g % NBANK][:]
        if g % 2 == 0:
            nc.vector.wait_ge(s_mm, g + 1)
            nc.vector.tensor_scalar_mul(dst, src, EVICT).then_inc(s_evv, 1)
        else:
            nc.scalar.wait_ge(s_mm, g + 1)
            nc.scalar.activation(
                dst, src, mybir.ActivationFunctionType.Copy, scale=EVICT
            ).then_inc(s_evs, 1)

    def oview(b):
        return out_d[b].rearrange("(c p) t f -> p c (t f)", p=128)

    # ---- sync: stores (HWDGE: completion sems post fast), in readiness
    # order; b3 split in halves so the post-stream tail is short ----
    for evv, evs, dst, srcv in (
        (6, 6, oview(0), ots[0][:]),
        (12, 12, oview(1), ots[1][:]),
        (18, 18, oview(2), ots[2][:]),
        (21, 21, oview(3)[:, 0:2], ots[3][:, 0:2]),
        (24, 24, oview(3)[:, 2:4], ots[3][:, 2:4]),
    ):
        nc.sync.wait_ge(s_evv, evv)
        nc.sync.wait_ge(s_evs, evs)
        nc.sync.dma_start(dst, srcv).then_inc(s_st, 16)

    # ---- end: clear sems for re-execution safety once all eviction /
    # matmul / load sem traffic has retired (evv/evs full counts imply
    # everything upstream). Store DMAs carry no sem: the NEFF epilogue's
    # own ring drain covers them, overlapped with the final wire. ----
    nc.gpsimd.wait_ge(s_evv, 24)
    nc.gpsimd.wait_ge(s_evs, 24)
    nc.clear_and_free_semaphores(s_lds + [s_mm, s_evv, s_evs])

    nc.compile()
    return nc


def prepare(x, adj, alpha, w, d, w2, d2):
    """Host prep: fold parameters, build q. Returns (nc, in_maps)."""
    import ml_dtypes

    x = np.ascontiguousarray(np.asarray(x), np.float32)
    adj = np.asarray(adj)
    alpha = np.asarray(alpha)
    w = np.asarray(w)
    d = np.asarray(d)
    w2 = np.asarray(w2)
    d2 = np.asarray(d2)
    a = 1.0 / (1.0 + np.exp(-alpha.astype(np.float32)))
    A = 0.125 * a[:, None] * adj.astype(np.float32)
    at = np.ascontiguousarray(
        np.clip(A.T * SCALE_AT, -240.0, 240.0)).astype(ml_dtypes.float8_e4m3)

    dc = np.clip(d.astype(np.float32), 0.0, 1.0)
    W = (w.astype(np.float32) * dc) @ w.astype(np.float32).T
    R = W.sum(axis=1)  # [FA]
    d2c = np.clip(d2.astype(np.float32), 0.0, 1.0)
    W2 = (w2.astype(np.float32) * d2c) @ w2.astype(np.float32).T  # [T,T]

    S = x.sum(axis=3)  # [B,N,T]

    # q = 0.5*x + 0.25*(x @_t W2) + 0.25*S*R[:64], kept in host fp32
    q = np.matmul(x.transpose(0, 1, 3, 2), 0.25 * W2).transpose(0, 1, 3, 2)
    q += 0.5 * x
    q += 0.25 * S[..., None] * R[:F]
    xb = x.astype(ml_dtypes.float8_e4m3)

    if "nc" not in _CACHE:
        _CACHE["nc"] = _build()
    nc = _CACHE["nc"]
    in_maps = [
        {"xin": xb[c * BPC:(c + 1) * BPC], "at": at}
        for c in range(N_CORES)
    ]
    _CACHE["q"] = q
    # host-side rank-1 pad columns: relu(0.25 * S * R[64:74])
    _CACHE["pad"] = np.maximum(
        0.25 * S[..., None] * R[F:], 0.0).astype(np.float32)
    return nc, in_maps


def _assemble(results):
    out = np.empty((B, N, T, FA), np.float32)
    adev = np.concatenate(
        [np.asarray(results[c]["out"]) for c in range(N_CORES)], axis=0)
    out[..., :F] = np.maximum(
        _CACHE["q"] + adev.astype(np.float32) * (1.0 / SCALE_A), 0.0)
    out[..., F:] = _CACHE["pad"]
    return out


def kernel(x, adj, alpha, w, d, w2, d2):
    from concourse.bass_utils import run_bass_kernel_spmd

    nc, in_maps = prepare(x, adj, alpha, w, d, w2, d2)
    res = run_bass_kernel_spmd(nc, in_maps, list(range(N_CORES)))
    return _assemble(res.results)
